# revision 3
# baseline (speedup 1.0000x reference)
"""EuclideanCodebook (VQ) kernel for 8 Trainium2 NeuronCores.

Strategy (data-parallel, mirrors sharding_hint):
  - x [8, 32768, 128] is sharded along tokens: core c takes batch c (32768 tokens).
  - codebook embed [1024, 128] is replicated (passed pre-transposed as embT).
  - Each core computes s[n,k] = f.e - e2/2 (argmax-equivalent to the reference
    distance) via fp32 PE matmul, exact fp32 argmax via DVE max/max_index,
    and returns per-core embed_ind. Host gathers + reduces (EMA update).
"""
import sys
sys.path.insert(0, "/opt/trn_rl_repo")

import numpy as np
from contextlib import ExitStack

import concourse.bass as bass
import concourse.tile as tile
from concourse import bacc, mybir
from concourse import bass_utils

N_CORES = 8
D = 128
K = 1024
N_PER_CORE = 32768
TOK_TILE = 128
N_TILES = N_PER_CORE // TOK_TILE  # 256
DECAY = 0.8
EPS = 1e-7

FP32 = mybir.dt.float32
U16 = mybir.dt.uint16


def build_module(n_tiles: int):
    nc = bacc.Bacc("TRN2", target_bir_lowering=False, debug=False,
                   enable_asserts=False, num_devices=N_CORES)
    x_d = nc.dram_tensor("x", [n_tiles * TOK_TILE, D], FP32, kind="ExternalInput").ap()
    embT_d = nc.dram_tensor("embT", [D, K], FP32, kind="ExternalInput").ap()
    nege2h_d = nc.dram_tensor("nege2h", [128, K], FP32, kind="ExternalInput").ap()
    ident_d = nc.dram_tensor("ident", [128, 128], FP32, kind="ExternalInput").ap()
    ind_d = nc.dram_tensor("ind", [128, n_tiles], U16, kind="ExternalOutput").ap()

    with tile.TileContext(nc) as tc:
        with ExitStack() as ctx:
            const = ctx.enter_context(tc.tile_pool(name="const", bufs=1))
            xin = ctx.enter_context(tc.tile_pool(name="xin", bufs=4))
            xtp = ctx.enter_context(tc.tile_pool(name="xtp", bufs=3, space="PSUM"))
            xts = ctx.enter_context(tc.tile_pool(name="xts", bufs=3))
            fep = ctx.enter_context(tc.tile_pool(name="fep", bufs=2, space="PSUM"))
            ssb = ctx.enter_context(tc.tile_pool(name="ssb", bufs=3))
            small = ctx.enter_context(tc.tile_pool(name="small", bufs=4))

            embT = const.tile([D, K], FP32)
            nc.sync.dma_start(embT[:], embT_d[:])
            nege2h = const.tile([128, K], FP32)
            nc.sync.dma_start(nege2h[:], nege2h_d[:])
            ident = const.tile([128, 128], FP32)
            nc.sync.dma_start(ident[:], ident_d[:])
            ones8 = const.tile([128, 8], FP32)
            nc.vector.memset(ones8[:], 1.0)
            idx_acc = const.tile([128, n_tiles], U16)

            for i in range(n_tiles):
                # load x tile [128 tok, 128 d]
                xt = xin.tile([TOK_TILE, D], FP32)
                nc.sync.dma_start(xt[:], x_d[i * TOK_TILE:(i + 1) * TOK_TILE, :])
                # transpose -> [d, tok] (PSUM), evict to SBUF via ACT
                xT_ps = xtp.tile([D, TOK_TILE], FP32)
                nc.tensor.transpose(xT_ps[:], xt[:], ident[:])
                xT = xts.tile([D, TOK_TILE], FP32)
                nc.scalar.copy(xT[:], xT_ps[:])
                # scores: fe[tok, k] = xT.T @ embT
                fe = fep.tile([TOK_TILE, K], FP32)
                nc.tensor.matmul(fe[:, 0:512], xT[:], embT[:, 0:512], start=True, stop=True)
                nc.tensor.matmul(fe[:, 512:1024], xT[:], embT[:, 512:1024], start=True, stop=True)
                # s = fe + (-e2/2), m = rowmax(s)
                s = ssb.tile([TOK_TILE, K], FP32)
                m = small.tile([TOK_TILE, 1], FP32)
                nc.vector.tensor_tensor(s[:], fe[:], nege2h[:], op=mybir.AluOpType.add)
                nc.vector.tensor_reduce(m[:], s[:], op=mybir.AluOpType.max,
                                        axis=mybir.AxisListType.X)
                # m8 = m replicated to 8 cols; idx8 = first indices of m in s
                m8 = small.tile([TOK_TILE, 8], FP32)
                nc.vector.tensor_scalar(m8[:], ones8[:], m[:], None,
                                        op0=mybir.AluOpType.mult)
                idx8 = small.tile([TOK_TILE, 8], U16)
                nc.vector.max_index(idx8[:], m8[:], s[:])
                nc.vector.tensor_copy(idx_acc[:, i:i + 1], idx8[:, 0:1])

            nc.sync.dma_start(ind_d[:], idx_acc[:])
    nc.compile()
    return nc


_NC_CACHE = {}


def _get_module(n_tiles=N_TILES):
    if n_tiles not in _NC_CACHE:
        _NC_CACHE[n_tiles] = build_module(n_tiles)
    return _NC_CACHE[n_tiles]


def _host_inputs(x, embed):
    flatten = np.ascontiguousarray(x.reshape(-1, D).astype(np.float32))
    embT = np.ascontiguousarray(embed.T.astype(np.float32))          # [D, K]
    e2 = (embed.astype(np.float32) ** 2).sum(axis=1)                  # [K]
    nege2h = np.ascontiguousarray(
        np.broadcast_to((-0.5 * e2)[None, :], (128, K))).astype(np.float32)
    ident = np.eye(128, dtype=np.float32)
    return flatten, embT, nege2h, ident


def kernel(x, embed, ema_embed, ema_num):
    x = np.asarray(x); embed = np.asarray(embed)
    ema_embed = np.asarray(ema_embed); ema_num = np.asarray(ema_num)
    nc = _get_module()
    flatten, embT, nege2h, ident = _host_inputs(x, embed)
    in_maps = []
    for c in range(N_CORES):
        in_maps.append({
            "x": flatten[c * N_PER_CORE:(c + 1) * N_PER_CORE],
            "embT": embT, "nege2h": nege2h, "ident": ident,
        })
    res = bass_utils.run_bass_kernel_spmd(nc, in_maps, core_ids=list(range(N_CORES)))
    # gather/unshard
    ind = np.concatenate(
        [r["ind"].T.reshape(-1) for r in res.results]).astype(np.int64)  # [N]
    quantize = embed[ind].reshape(x.shape)
    counts = np.bincount(ind, minlength=K).astype(np.float32)
    sums = np.zeros((K, D), np.float32)
    np.add.at(sums, ind, flatten)
    ema_num_new = DECAY * ema_num + (1.0 - DECAY) * counts
    ema_embed_new = DECAY * ema_embed + (1.0 - DECAY) * sums
    total = ema_num_new.sum()
    smoothed = (ema_num_new + EPS) / (total + K * EPS) * total
    embed_new = ema_embed_new / smoothed[:, None]
    return quantize, embed_new, ema_num_new, ema_embed_new
